# revision 13
# baseline (speedup 1.0000x reference)
"""Fused GQA attention block (RoPE + causal softmax + o-proj) on 8 trn2 cores.

Sharding: core = b*4 + g  (b = batch, g = head-group).  Each core gets batch
b's x (pre-transposed to [C, T] on host), 4 query heads + 1 KV head of the
projection weights, and the matching 512 rows of Wo.  It returns the partial
o = Y_g @ Wo_g; the host sums the 4 partials per batch (row-parallel o-proj
with the reduction done at gather time).

All matmuls run in float32r (TF32-like) at bf16 speed.  Attention is computed
in S^T layout ([k-block, q] tiles) so no P transpose is needed; softmax
denominators come from one-hot column matmuls accumulated per q-tile.
"""

import numpy as np
from contextlib import ExitStack

B, T, C = 2, 2048, 2048
NH, NKV, HD = 16, 4, 128
ROPE_BASE = 10000.0
NCORES = 8
HPG = NH // NKV          # 4 query heads per core
P = 128
KSUB = C // P            # 16 contraction subtiles
CHW = 512                # qkv chunk width (moving N)
NCH = T // CHW           # 4 chunks
QKO = 4                  # ko quarters per xt tile
TQ = 512                 # attention q tile
NQT = T // TQ            # 4 q tiles
NKB = T // P             # 16 k blocks
SCALE = float(1.0 / np.sqrt(np.float32(HD)))
NEG = -1.0e30

_CACHE = {}


def _build(**opt):
    opt.setdefault("sps_bufs", 4)
    opt.setdefault("spw", 1)
    opt.setdefault("yps_bufs", 1)
    opt.setdefault("dp_bufs", 1)
    opt.setdefault("vt_tag", "dp")
    import concourse.tile as tile
    from concourse import bacc, mybir

    f32 = mybir.dt.float32
    f32r = mybir.dt.float32r
    Exp = mybir.ActivationFunctionType.Exp

    nc = bacc.Bacc("TRN2", target_bir_lowering=False, debug=False)

    xt_d = nc.dram_tensor("xt", [C, T], f32, kind="ExternalInput").ap()
    wq_d = nc.dram_tensor("wq", [C, HPG * HD], f32, kind="ExternalInput").ap()
    wk_d = nc.dram_tensor("wk", [C, HD], f32, kind="ExternalInput").ap()
    wv_d = nc.dram_tensor("wv", [C, HD], f32, kind="ExternalInput").ap()
    wo_d = nc.dram_tensor("wo", [HPG * HD, C], f32, kind="ExternalInput").ap()
    cosf_d = nc.dram_tensor("cosf", [P, T], f32, kind="ExternalInput").ap()
    sinf_d = nc.dram_tensor("sinf", [P, T], f32, kind="ExternalInput").ap()
    mtri_d = nc.dram_tensor("mtri", [P, P], f32, kind="ExternalInput").ap()
    ohd_d = nc.dram_tensor("ohd", [P, HPG * P], f32, kind="ExternalInput").ap()
    ohbc_d = nc.dram_tensor("ohbc", [HPG, HPG * P], f32, kind="ExternalInput").ap()
    idn_d = nc.dram_tensor("idn", [P, P], f32, kind="ExternalInput").ap()
    out_d = nc.dram_tensor("o_part", [T, C], f32, kind="ExternalOutput").ap()

    with tile.TileContext(nc) as tc, ExitStack() as ctx:
        const = ctx.enter_context(tc.tile_pool(name="const", bufs=1))
        wpool = ctx.enter_context(tc.tile_pool(name="wpool", bufs=1))
        xpool = ctx.enter_context(tc.tile_pool(name="xpool", bufs=5))
        qk = ctx.enter_context(tc.tile_pool(name="qk", bufs=1))
        sw = ctx.enter_context(tc.tile_pool(name="sw", bufs=2))
        ptp = ctx.enter_context(tc.tile_pool(name="ptp", bufs=3))
        yp = ctx.enter_context(tc.tile_pool(name="yp", bufs=2))
        op = ctx.enter_context(tc.tile_pool(name="op", bufs=2))
        psum = ctx.enter_context(tc.tile_pool(name="psum", bufs=1, space="PSUM"))

        # ---- interleaved startup DMAs: K/V weights + first x quarters first,
        # then Wq per head, so K/V projections start within ~8us ----
        xt4 = xt_d.rearrange("(kq ko ki) t -> ki kq ko t", ki=P, ko=QKO).bitcast(f32r)

        def load_xt_quarters(ch, skip_dma=False):
            c0 = ch * CHW
            qs = []
            for kq in range(KSUB // QKO):
                xq = xpool.tile([P, QKO, CHW], f32r, tag="xt", name=f"xt_{ch}_{kq}")
                if not skip_dma:
                    nc.sync.dma_start(xq[:], xt4[:, kq, :, c0:c0 + CHW])
                qs.append(xq)
            return qs

        wk = wpool.tile([P, KSUB, HD], f32r)
        wv = wpool.tile([P, KSUB, HD], f32r)
        wq = wpool.tile([P, KSUB, HPG * HD], f32r, tag="wbig")
        wq3 = wq_d.rearrange("(ko ki) m -> ki ko m", ki=P).bitcast(f32r)

        xtq0 = load_xt_quarters(0, skip_dma=True)
        wk3 = wk_d.rearrange("(ko ki) m -> ki ko m", ki=P).bitcast(f32r)
        wv3 = wv_d.rearrange("(ko ki) m -> ki ko m", ki=P).bitcast(f32r)
        # interleave ko-chunked weight loads with x quarters: first K matmuls
        # need only wk[:, 0:4] + xtq0[0]
        nc.sync.dma_start(wk[:, 0:4], wk3[:, 0:4])
        nc.sync.dma_start(xtq0[0][:], xt4[:, 0, :, 0:CHW])
        nc.sync.dma_start(wk[:, 4:8], wk3[:, 4:8])
        nc.sync.dma_start(wk[:, 8:16], wk3[:, 8:16])
        nc.sync.dma_start(xtq0[1][:], xt4[:, 1, :, 0:CHW])
        nc.sync.dma_start(wv[:, 0:8], wv3[:, 0:8])
        nc.sync.dma_start(wv[:, 8:16], wv3[:, 8:16])
        nc.sync.dma_start(xtq0[2][:], xt4[:, 2, :, 0:CHW])
        nc.sync.dma_start(wq[:, :, 0:HD], wq3[:, :, 0:HD])
        nc.sync.dma_start(xtq0[3][:], xt4[:, 3, :, 0:CHW])
        nc.sync.dma_start(wq[:, :, HD:2 * HD], wq3[:, :, HD:2 * HD])
        nc.sync.dma_start(wq[:, :, 2 * HD:3 * HD], wq3[:, :, 2 * HD:3 * HD])
        nc.sync.dma_start(wq[:, :, 3 * HD:4 * HD], wq3[:, :, 3 * HD:4 * HD])

        # ---- constants (needed from rope/attention onwards) ----
        cosf = const.tile([P, T], f32r)
        sinf = const.tile([P, T], f32r)
        mtri = const.tile([P, P], f32)
        ohd = const.tile([P, HPG, P], f32r)
        ohbc = const.tile([HPG, HPG, P], f32r)
        idn = const.tile([P, P], f32r)
        nc.sync.dma_start(cosf[:], cosf_d.bitcast(f32r))
        nc.sync.dma_start(sinf[:], sinf_d.bitcast(f32r))
        nc.sync.dma_start(mtri[:], mtri_d)
        nc.sync.dma_start(ohd[:], ohd_d.rearrange("p (h m) -> p h m", h=HPG).bitcast(f32r))
        nc.sync.dma_start(ohbc[:], ohbc_d.rearrange("p (h m) -> p h m", h=HPG).bitcast(f32r))
        nc.sync.dma_start(idn[:], idn_d.bitcast(f32r))

        # ---- persistent activations ----
        qsb = qk.tile([P, HPG, T], f32r)      # Q^T per head, roped in place
        ksb = qk.tile([P, T], f32r)           # K^T, roped in place
        vsb = qk.tile([P, NKB, HD], f32r)     # V ([k-block, hd] blocks)


        SWAPMASK = [(i ^ 1) for i in range(32)]

        def rope(dst_sl, sl):
            w = sw.tile([P, CHW], f32r, tag="swap", name="ropew")
            nc.vector.stream_shuffle(w.bitcast(f32), dst_sl.bitcast(f32), SWAPMASK)
            nc.vector.tensor_mul(w[:], w[:], sinf[:, sl])
            nc.vector.tensor_mul(dst_sl, dst_sl, cosf[:, sl])
            nc.vector.tensor_add(dst_sl, dst_sl, w[:])

        # ================= QKV projections =================
        for ch in range(NCH):
            c0 = ch * CHW
            xtq = xtq0 if ch == 0 else load_xt_quarters(ch)

            def proj(west, m0, m1, psname):
                ps = psum.tile([P, CHW], f32, tag="mm", bufs=2, name=psname)
                for ko in range(KSUB):
                    nc.tensor.matmul(ps[:], west[:, ko, m0:m1], xtq[ko // QKO][:, ko % QKO, :],
                                     start=(ko == 0), stop=(ko == KSUB - 1))
                return ps

            kps = proj(wk, 0, HD, f"kps_{ch}")
            nc.scalar.copy(ksb[:, c0:c0 + CHW], kps[:])
            rope(ksb[:, c0:c0 + CHW], slice(c0, c0 + CHW))

            vps = proj(wv, 0, HD, f"vps_{ch}")
            vtc = sw.tile([P, CHW], f32r, tag="vtc")
            nc.scalar.copy(vtc[:], vps[:])
            # V^T -> V per 128-block via PE transpose
            for i in range(CHW // P):
                kb = (c0 + i * P) // P
                vtp = psum.tile([P, P], f32r, tag=opt.get("vt_tag","yps"), bufs=opt.get("yps_bufs", 2) if opt.get("vt_tag","yps")=="yps" else opt.get("dp_bufs", 2), name=f"vtp_{kb}")
                nc.tensor.transpose(vtp[:], vtc[:, i * P:(i + 1) * P], idn[:])
                nc.vector.tensor_copy(vsb[:, kb, :], vtp[:])

            for h in range(HPG):
                qps = proj(wq, h * HD, (h + 1) * HD, f"qps_{ch}_{h}")
                nc.scalar.copy(qsb[:, h, c0:c0 + CHW], qps[:])
                rope(qsb[:, h, c0:c0 + CHW], slice(c0, c0 + CHW))


        # ---- Wo loads into the slot wq frees ----
        wo = wpool.tile([P, HPG, C], f32r, tag="wbig")
        wo3 = wo_d.rearrange("(h ki) c -> ki h c", ki=P).bitcast(f32r)
        for _h in range(HPG):
            nc.sync.dma_start(wo[:, _h, :], wo3[:, _h, :])

        # ================= attention (+ software-pipelined o-proj) ==========
        ysbs = {}

        def attention(j, pipelined=False):
            q0 = j * TQ
            nkb = 4 * j + 4   # causal: k blocks 0..4j+3
            dps = psum.tile([P, TQ], f32, tag="dp", bufs=opt.get("dp_bufs", 2), name=f"dps_{j}")
            ysb = yp.tile([P, HPG, TQ], f32r, tag="ysb", name=f"ysb_{j}")
            ysbs[j] = ysb

            SPW = opt.get('spw', 2)
            for h in range(HPG):
                yps = psum.tile([P, TQ], f32, tag="yps", bufs=opt.get("yps_bufs", 2), name=f"yps_{j}_{h}")
                for pi in range((nkb + SPW - 1) // SPW):
                    sps = psum.tile([P, SPW, TQ], f32, tag="sps", bufs=opt.get("sps_bufs", 1),
                                    name=f"sps_{j}_{h}_{pi}")
                    pt = ptp.tile([P, SPW, TQ], f32r, tag="pt", name=f"pt_{j}_{h}_{pi}", bufs=opt.get("pt_bufs", 3))
                    for i in range(SPW):
                        kb = SPW * pi + i
                        r = kb - 4 * j
                        if r < 0:  # fully causal tile
                            nc.tensor.matmul(sps[:, i, :], ksb[:, kb * P:(kb + 1) * P],
                                             qsb[:, h, q0:q0 + TQ], start=True, stop=True)
                        else:      # diagonal: only cols [128r, 512) are live
                            m0 = P * r
                            nc.tensor.matmul(sps[:, i, m0:TQ], ksb[:, kb * P:(kb + 1) * P],
                                             qsb[:, h, q0 + m0:q0 + TQ], start=True, stop=True)
                            nc.vector.tensor_add(sps[:, i, m0:m0 + P],
                                                 sps[:, i, m0:m0 + P], mtri[:])
                    if SPW * pi + SPW - 1 < 4 * j:
                        nc.scalar.activation(pt[:, :, :], sps[:, :, :], Exp, scale=SCALE)
                    else:
                        for i in range(SPW):
                            kb = SPW * pi + i
                            m0 = P * max(0, kb - 4 * j)
                            nc.scalar.activation(pt[:, i, m0:TQ], sps[:, i, m0:TQ],
                                                 Exp, scale=SCALE)
                    for i in range(SPW):
                        kb = SPW * pi + i
                        m0 = P * max(0, kb - 4 * j)
                        nc.tensor.matmul(yps[:, m0:TQ], vsb[:, kb, :], pt[:, i, m0:TQ],
                                         start=(kb == 0), stop=(kb == nkb - 1),
                                         skip_group_check=True)
                        nc.tensor.matmul(dps[:, m0:TQ], ohd[:, h, :], pt[:, i, m0:TQ],
                                         start=(h == 0 and kb == 0),
                                         stop=(h == HPG - 1 and kb == nkb - 1),
                                         skip_group_check=True)
                nc.vector.tensor_copy(ysb[:, h, :], yps[:])
                if pipelined:
                    oproj_tb(j - 1, h)

            # softmax denominators -> reciprocal -> broadcast -> scale Y^T
            if pipelined:
                ysbs.pop(j - 1)
            rec = ptp.tile([HPG, TQ], f32r, tag="rec", bufs=2, name=f"rec_{j}")
            with nc.allow_low_precision(reason="f32r is fp32-width"):
                nc.vector.reciprocal(rec[:], dps[0:HPG, :])
            for h in range(HPG):
                rbc = psum.tile([P, TQ], f32, tag="mm", bufs=2, name=f"rbc_{j}_{h}")
                nc.tensor.matmul(rbc[:], ohbc[:, h, :], rec[:], start=True, stop=True)
                nc.vector.tensor_mul(ysb[:, h, :], ysb[:, h, :], rbc[:])

        OSPL = opt.get('ospl', 2)

        def oproj_tb(j, tb):
            q0 = j * TQ
            ysb = ysbs[j]
            if True:
                t0 = tb * P
                for half in range(OSPL):
                    osb = op.tile([P, C // OSPL], f32, tag="osb", name=f"osb_{j}_{tb}_{half}")
                    for cq in range(4 // OSPL):
                        ct = half * (4 // OSPL) + cq
                        ops = psum.tile([P, TQ], f32, tag="mm", bufs=2,
                                        name=f"ops_{j}_{tb}_{ct}")
                        for h in range(HPG):
                            nc.tensor.matmul(ops[:], ysb[:, h, t0:t0 + P],
                                             wo[:, h, ct * TQ:(ct + 1) * TQ],
                                             start=(h == 0), stop=(h == HPG - 1))
                        nc.vector.tensor_copy(osb[:, cq * TQ:(cq + 1) * TQ], ops[:])
                    nc.sync.dma_start(
                        out_d[q0 + t0:q0 + t0 + P, half * (C // OSPL):(half + 1) * (C // OSPL)],
                        osb[:])

        def oproj(j):
            for tb in range(TQ // P):
                oproj_tb(j, tb)
            ysbs.pop(j)

        for j in range(NQT):
            attention(j, pipelined=(j >= 1))
        oproj(NQT - 1)

    nc.compile()
    return nc


def _host_inputs(x, rope_cache, Wq, Wk, Wv, Wo):
    x = np.asarray(x, dtype=np.float32)
    rope_cache = np.asarray(rope_cache, dtype=np.float32)
    Wq = np.asarray(Wq, dtype=np.float32)
    Wk = np.asarray(Wk, dtype=np.float32)
    Wv = np.asarray(Wv, dtype=np.float32)
    Wo = np.asarray(Wo, dtype=np.float32)

    # interleaved pair layout: partitions (2i, 2i+1) hold pair i; the rope
    # swap is then an adjacent-partition stream_shuffle on the DVE
    cos = rope_cache[:T, :, 0].T.astype(np.float32)   # [64, T]
    sin = rope_cache[:T, :, 1].T.astype(np.float32)
    cosf = np.empty((P, T), np.float32)
    sinf = np.empty((P, T), np.float32)
    cosf[0::2] = cos
    cosf[1::2] = cos
    sinf[0::2] = -sin
    sinf[1::2] = sin
    cosf = np.ascontiguousarray(cosf)
    sinf = np.ascontiguousarray(sinf)

    mtri = np.where(np.arange(P)[:, None] <= np.arange(P)[None, :], 0.0, NEG
                    ).astype(np.float32)
    # ohd[k, h, m] = (m == h): one-hot column matmul lands head-h denominators
    # on psum partition h (M=128 so it stays in the fast matmul class)
    ohd = np.zeros((P, HPG, P), np.float32)
    for h in range(HPG):
        ohd[:, h, h] = 1.0
    ohd = ohd.reshape(P, HPG * P)
    ohbc = np.zeros((HPG, HPG, P), np.float32)
    for h in range(HPG):
        ohbc[h, h, :] = 1.0
    ohbc = ohbc.reshape(HPG, HPG * P)
    idn = np.eye(P, dtype=np.float32)

    in_maps = []
    for core in range(NCORES):
        b, g = divmod(core, HPG)
        xt = np.ascontiguousarray(x[b].T)
        wq = np.ascontiguousarray(Wq[:, g * HPG * HD:(g + 1) * HPG * HD])
        wk = np.ascontiguousarray(Wk[:, g * HD:(g + 1) * HD])
        wv = np.ascontiguousarray(Wv[:, g * HD:(g + 1) * HD])
        wo = np.ascontiguousarray(Wo[g * HPG * HD:(g + 1) * HPG * HD, :])
        in_maps.append({
            "xt": xt, "wq": wq, "wk": wk, "wv": wv, "wo": wo,
            "cosf": cosf, "sinf": sinf, "mtri": mtri,
            "ohd": ohd, "ohbc": ohbc, "idn": idn,
        })
    return in_maps


def run(x, rope_cache, Wq, Wk, Wv, Wo, trace=False, **kw):
    from concourse.bass_utils import run_bass_kernel_spmd

    if "nc" not in _CACHE:
        _CACHE["nc"] = _build()
    nc = _CACHE["nc"]
    in_maps = _host_inputs(x, rope_cache, Wq, Wk, Wv, Wo)
    res = run_bass_kernel_spmd(nc, in_maps, core_ids=list(range(NCORES)),
                               trace=trace, **kw)
    out = np.empty((B, T, C), np.float32)
    for b in range(B):
        acc = res.results[b * HPG]["o_part"].astype(np.float32).copy()
        for g in range(1, HPG):
            acc += res.results[b * HPG + g]["o_part"]
        out[b] = acc
    return out, res


def kernel(x, rope_cache, Wq, Wk, Wv, Wo):
    out, _ = run(x, rope_cache, Wq, Wk, Wv, Wo, trace=False)
    return out


# revision 15
# speedup vs baseline: 1.0060x; 1.0060x over previous
"""Fused GQA attention block (RoPE + causal softmax + o-proj) on 8 trn2 cores.

Sharding: core = b*4 + g  (b = batch, g = head-group).  Each core gets batch
b's x (pre-transposed to [C, T] on host), 4 query heads + 1 KV head of the
projection weights, and the matching 512 rows of Wo.  It returns the partial
o = Y_g @ Wo_g; the host sums the 4 partials per batch (row-parallel o-proj
with the reduction done at gather time).

All matmuls run in float32r (TF32-like) at bf16 speed.  Attention is computed
in S^T layout ([k-block, q] tiles) so no P transpose is needed; softmax
denominators come from one-hot column matmuls accumulated per q-tile.
"""

import numpy as np
from contextlib import ExitStack

B, T, C = 2, 2048, 2048
NH, NKV, HD = 16, 4, 128
ROPE_BASE = 10000.0
NCORES = 8
HPG = NH // NKV          # 4 query heads per core
P = 128
KSUB = C // P            # 16 contraction subtiles
CHW = 512                # qkv chunk width (moving N)
NCH = T // CHW           # 4 chunks
QKO = 4                  # ko quarters per xt tile
TQ = 512                 # attention q tile
NQT = T // TQ            # 4 q tiles
NKB = T // P             # 16 k blocks
SCALE = float(1.0 / np.sqrt(np.float32(HD)))
NEG = -1.0e30

_CACHE = {}


def _build(**opt):
    opt.setdefault("spw", 1)
    opt.setdefault("sps_bufs", 3)
    opt.setdefault("yps_bufs", 1)
    opt.setdefault("dp_bufs", 2)
    opt.setdefault("vt_tag", "dp")
    import concourse.tile as tile
    from concourse import bacc, mybir

    f32 = mybir.dt.float32
    f32r = mybir.dt.float32r
    Exp = mybir.ActivationFunctionType.Exp

    nc = bacc.Bacc("TRN2", target_bir_lowering=False, debug=False)

    xt_d = nc.dram_tensor("xt", [C, T], f32, kind="ExternalInput").ap()
    wq_d = nc.dram_tensor("wq", [C, HPG * HD], f32, kind="ExternalInput").ap()
    wk_d = nc.dram_tensor("wk", [C, HD], f32, kind="ExternalInput").ap()
    wv_d = nc.dram_tensor("wv", [C, HD], f32, kind="ExternalInput").ap()
    wo_d = nc.dram_tensor("wo", [HPG * HD, C], f32, kind="ExternalInput").ap()
    cosf_d = nc.dram_tensor("cosf", [P, T], f32, kind="ExternalInput").ap()
    sinf_d = nc.dram_tensor("sinf", [P, T], f32, kind="ExternalInput").ap()
    mtri_d = nc.dram_tensor("mtri", [P, P], f32, kind="ExternalInput").ap()
    ohd_d = nc.dram_tensor("ohd", [P, HPG * P], f32, kind="ExternalInput").ap()
    ohbc_d = nc.dram_tensor("ohbc", [HPG, HPG * P], f32, kind="ExternalInput").ap()
    idn_d = nc.dram_tensor("idn", [P, P], f32, kind="ExternalInput").ap()
    out_d = nc.dram_tensor("o_part", [T, C], f32, kind="ExternalOutput").ap()

    with tile.TileContext(nc) as tc, ExitStack() as ctx:
        const = ctx.enter_context(tc.tile_pool(name="const", bufs=1))
        wpool = ctx.enter_context(tc.tile_pool(name="wpool", bufs=1))
        xpool = ctx.enter_context(tc.tile_pool(name="xpool", bufs=5))
        qk = ctx.enter_context(tc.tile_pool(name="qk", bufs=1))
        sw = ctx.enter_context(tc.tile_pool(name="sw", bufs=2))
        ptp = ctx.enter_context(tc.tile_pool(name="ptp", bufs=3))
        yp = ctx.enter_context(tc.tile_pool(name="yp", bufs=2))
        op = ctx.enter_context(tc.tile_pool(name="op", bufs=2))
        psum = ctx.enter_context(tc.tile_pool(name="psum", bufs=1, space="PSUM"))

        # ---- interleaved startup DMAs: K/V weights + first x quarters first,
        # then Wq per head, so K/V projections start within ~8us ----
        xt4 = xt_d.rearrange("(kq ko ki) t -> ki kq ko t", ki=P, ko=QKO).bitcast(f32r)

        def load_xt_quarters(ch, skip_dma=False):
            c0 = ch * CHW
            qs = []
            for kq in range(KSUB // QKO):
                xq = xpool.tile([P, QKO, CHW], f32r, tag="xt", name=f"xt_{ch}_{kq}")
                if not skip_dma:
                    nc.sync.dma_start(xq[:], xt4[:, kq, :, c0:c0 + CHW])
                qs.append(xq)
            return qs

        wk = wpool.tile([P, KSUB, HD], f32r)
        wv = wpool.tile([P, KSUB, HD], f32r)
        wq = wpool.tile([P, KSUB, HPG * HD], f32r, tag="wbig")
        wq3 = wq_d.rearrange("(ko ki) m -> ki ko m", ki=P).bitcast(f32r)

        xtq0 = load_xt_quarters(0, skip_dma=True)
        wk3 = wk_d.rearrange("(ko ki) m -> ki ko m", ki=P).bitcast(f32r)
        wv3 = wv_d.rearrange("(ko ki) m -> ki ko m", ki=P).bitcast(f32r)
        # interleave ko-chunked weight loads with x quarters: first K matmuls
        # need only wk[:, 0:4] + xtq0[0]
        nc.sync.dma_start(wk[:, 0:4], wk3[:, 0:4])
        nc.sync.dma_start(xtq0[0][:], xt4[:, 0, :, 0:CHW])
        nc.sync.dma_start(wk[:, 4:8], wk3[:, 4:8])
        nc.sync.dma_start(wk[:, 8:16], wk3[:, 8:16])
        nc.sync.dma_start(xtq0[1][:], xt4[:, 1, :, 0:CHW])
        nc.sync.dma_start(wv[:, 0:8], wv3[:, 0:8])
        nc.sync.dma_start(wv[:, 8:16], wv3[:, 8:16])
        nc.sync.dma_start(xtq0[2][:], xt4[:, 2, :, 0:CHW])
        nc.sync.dma_start(wq[:, :, 0:HD], wq3[:, :, 0:HD])
        nc.sync.dma_start(xtq0[3][:], xt4[:, 3, :, 0:CHW])
        nc.sync.dma_start(wq[:, :, HD:2 * HD], wq3[:, :, HD:2 * HD])
        nc.sync.dma_start(wq[:, :, 2 * HD:3 * HD], wq3[:, :, 2 * HD:3 * HD])
        nc.sync.dma_start(wq[:, :, 3 * HD:4 * HD], wq3[:, :, 3 * HD:4 * HD])

        # ---- constants (needed from rope/attention onwards) ----
        cosf = const.tile([P, T], f32r)
        sinf = const.tile([P, T], f32r)
        mtri = const.tile([P, P], f32)
        ohd = const.tile([P, HPG, P], f32r)
        ohbc = const.tile([HPG, HPG, P], f32r)
        idn = const.tile([P, P], f32r)
        nc.sync.dma_start(cosf[:], cosf_d.bitcast(f32r))
        nc.sync.dma_start(sinf[:], sinf_d.bitcast(f32r))
        nc.sync.dma_start(mtri[:], mtri_d)
        nc.sync.dma_start(ohd[:], ohd_d.rearrange("p (h m) -> p h m", h=HPG).bitcast(f32r))
        nc.sync.dma_start(ohbc[:], ohbc_d.rearrange("p (h m) -> p h m", h=HPG).bitcast(f32r))
        nc.sync.dma_start(idn[:], idn_d.bitcast(f32r))

        # ---- persistent activations ----
        qsb = qk.tile([P, HPG, T], f32r)      # Q^T per head, roped in place
        ksb = qk.tile([P, T], f32r)           # K^T, roped in place
        vsb = qk.tile([P, NKB, HD], f32r)     # V ([k-block, hd] blocks)


        SWAPMASK = [(i ^ 1) for i in range(32)]

        def rope(dst_sl, sl):
            w = sw.tile([P, CHW], f32r, tag="swap", name="ropew")
            nc.vector.stream_shuffle(w.bitcast(f32), dst_sl.bitcast(f32), SWAPMASK)
            nc.vector.tensor_mul(w[:], w[:], sinf[:, sl])
            nc.vector.tensor_mul(dst_sl, dst_sl, cosf[:, sl])
            nc.vector.tensor_add(dst_sl, dst_sl, w[:])

        # ================= QKV projections =================
        for ch in range(NCH):
            c0 = ch * CHW
            xtq = xtq0 if ch == 0 else load_xt_quarters(ch)

            def proj(west, m0, m1, psname):
                ps = psum.tile([P, CHW], f32, tag="mm", bufs=2, name=psname)
                for ko in range(KSUB):
                    nc.tensor.matmul(ps[:], west[:, ko, m0:m1], xtq[ko // QKO][:, ko % QKO, :],
                                     start=(ko == 0), stop=(ko == KSUB - 1))
                return ps

            kps = proj(wk, 0, HD, f"kps_{ch}")
            nc.scalar.copy(ksb[:, c0:c0 + CHW], kps[:])
            rope(ksb[:, c0:c0 + CHW], slice(c0, c0 + CHW))

            vps = proj(wv, 0, HD, f"vps_{ch}")
            vtc = sw.tile([P, CHW], f32r, tag="vtc")
            nc.scalar.copy(vtc[:], vps[:])
            # V^T -> V per 128-block via PE transpose
            for i in range(CHW // P):
                kb = (c0 + i * P) // P
                vtp = psum.tile([P, P], f32r, tag=opt.get("vt_tag","yps"), bufs=opt.get("yps_bufs", 2) if opt.get("vt_tag","yps")=="yps" else opt.get("dp_bufs", 2), name=f"vtp_{kb}")
                nc.tensor.transpose(vtp[:], vtc[:, i * P:(i + 1) * P], idn[:])
                nc.vector.tensor_copy(vsb[:, kb, :], vtp[:])

            for h in range(HPG):
                qps = proj(wq, h * HD, (h + 1) * HD, f"qps_{ch}_{h}")
                nc.scalar.copy(qsb[:, h, c0:c0 + CHW], qps[:])
                rope(qsb[:, h, c0:c0 + CHW], slice(c0, c0 + CHW))


        # ---- Wo loads into the slot wq frees ----
        wo = wpool.tile([P, HPG, C], f32r, tag="wbig")
        wo3 = wo_d.rearrange("(h ki) c -> ki h c", ki=P).bitcast(f32r)
        for _h in range(HPG):
            nc.sync.dma_start(wo[:, _h, :], wo3[:, _h, :])

        # ================= attention (+ software-pipelined o-proj) ==========
        ysbs = {}

        def attention(j, pipelined=False):
            q0 = j * TQ
            nkb = 4 * j + 4   # causal: k blocks 0..4j+3
            dps = psum.tile([P, TQ], f32, tag="dp", bufs=opt.get("dp_bufs", 2), name=f"dps_{j}")
            ysb = yp.tile([P, HPG, TQ], f32r, tag="ysb", name=f"ysb_{j}")
            ysbs[j] = ysb

            SPW = opt.get('spw', 2)
            for h in range(HPG):
                yps = psum.tile([P, TQ], f32, tag="yps", bufs=opt.get("yps_bufs", 2), name=f"yps_{j}_{h}")
                for pi in range((nkb + SPW - 1) // SPW):
                    sps = psum.tile([P, SPW, TQ], f32, tag="sps", bufs=opt.get("sps_bufs", 1),
                                    name=f"sps_{j}_{h}_{pi}")
                    pt = ptp.tile([P, SPW, TQ], f32r, tag="pt", name=f"pt_{j}_{h}_{pi}", bufs=opt.get("pt_bufs", 3))
                    for i in range(SPW):
                        kb = SPW * pi + i
                        r = kb - 4 * j
                        if r < 0:  # fully causal tile
                            nc.tensor.matmul(sps[:, i, :], ksb[:, kb * P:(kb + 1) * P],
                                             qsb[:, h, q0:q0 + TQ], start=True, stop=True)
                        else:      # diagonal: only cols [128r, 512) are live
                            m0 = P * r
                            nc.tensor.matmul(sps[:, i, m0:TQ], ksb[:, kb * P:(kb + 1) * P],
                                             qsb[:, h, q0 + m0:q0 + TQ], start=True, stop=True)
                            nc.vector.tensor_add(sps[:, i, m0:m0 + P],
                                                 sps[:, i, m0:m0 + P], mtri[:])
                    if SPW * pi + SPW - 1 < 4 * j:
                        nc.scalar.activation(pt[:, :, :], sps[:, :, :], Exp, scale=SCALE)
                    else:
                        for i in range(SPW):
                            kb = SPW * pi + i
                            m0 = P * max(0, kb - 4 * j)
                            nc.scalar.activation(pt[:, i, m0:TQ], sps[:, i, m0:TQ],
                                                 Exp, scale=SCALE)
                    for i in range(SPW):
                        kb = SPW * pi + i
                        m0 = P * max(0, kb - 4 * j)
                        nc.tensor.matmul(yps[:, m0:TQ], vsb[:, kb, :], pt[:, i, m0:TQ],
                                         start=(kb == 0), stop=(kb == nkb - 1),
                                         skip_group_check=True)
                        nc.tensor.matmul(dps[:, m0:TQ], ohd[:, h, :], pt[:, i, m0:TQ],
                                         start=(h == 0 and kb == 0),
                                         stop=(h == HPG - 1 and kb == nkb - 1),
                                         skip_group_check=True)
                nc.vector.tensor_copy(ysb[:, h, :], yps[:])
                if pipelined:
                    oproj_tb(j - 1, h)

            # softmax denominators -> reciprocal -> broadcast -> scale Y^T
            if pipelined:
                ysbs.pop(j - 1)
            rec = ptp.tile([HPG, TQ], f32r, tag="rec", bufs=2, name=f"rec_{j}")
            with nc.allow_low_precision(reason="f32r is fp32-width"):
                nc.vector.reciprocal(rec[:], dps[0:HPG, :])
            for h in range(HPG):
                rbc = psum.tile([P, TQ], f32, tag="mm", bufs=2, name=f"rbc_{j}_{h}")
                nc.tensor.matmul(rbc[:], ohbc[:, h, :], rec[:], start=True, stop=True)
                nc.vector.tensor_mul(ysb[:, h, :], ysb[:, h, :], rbc[:])

        OSPL = opt.get('ospl', 2)

        def oproj_tb(j, tb):
            q0 = j * TQ
            ysb = ysbs[j]
            if True:
                t0 = tb * P
                for half in range(OSPL):
                    osb = op.tile([P, C // OSPL], f32, tag="osb", name=f"osb_{j}_{tb}_{half}")
                    for cq in range(4 // OSPL):
                        ct = half * (4 // OSPL) + cq
                        ops = psum.tile([P, TQ], f32, tag="mm", bufs=2,
                                        name=f"ops_{j}_{tb}_{ct}")
                        for h in range(HPG):
                            nc.tensor.matmul(ops[:], ysb[:, h, t0:t0 + P],
                                             wo[:, h, ct * TQ:(ct + 1) * TQ],
                                             start=(h == 0), stop=(h == HPG - 1))
                        nc.vector.tensor_copy(osb[:, cq * TQ:(cq + 1) * TQ], ops[:])
                    nc.sync.dma_start(
                        out_d[q0 + t0:q0 + t0 + P, half * (C // OSPL):(half + 1) * (C // OSPL)],
                        osb[:])

        def oproj(j):
            for tb in range(TQ // P):
                oproj_tb(j, tb)
            ysbs.pop(j)

        for j in range(NQT):
            attention(j, pipelined=(j >= 1))
        oproj(NQT - 1)

    nc.compile()
    return nc


def _host_inputs(x, rope_cache, Wq, Wk, Wv, Wo):
    x = np.asarray(x, dtype=np.float32)
    rope_cache = np.asarray(rope_cache, dtype=np.float32)
    Wq = np.asarray(Wq, dtype=np.float32)
    Wk = np.asarray(Wk, dtype=np.float32)
    Wv = np.asarray(Wv, dtype=np.float32)
    Wo = np.asarray(Wo, dtype=np.float32)

    # interleaved pair layout: partitions (2i, 2i+1) hold pair i; the rope
    # swap is then an adjacent-partition stream_shuffle on the DVE
    cos = rope_cache[:T, :, 0].T.astype(np.float32)   # [64, T]
    sin = rope_cache[:T, :, 1].T.astype(np.float32)
    cosf = np.empty((P, T), np.float32)
    sinf = np.empty((P, T), np.float32)
    cosf[0::2] = cos
    cosf[1::2] = cos
    sinf[0::2] = -sin
    sinf[1::2] = sin
    cosf = np.ascontiguousarray(cosf)
    sinf = np.ascontiguousarray(sinf)

    mtri = np.where(np.arange(P)[:, None] <= np.arange(P)[None, :], 0.0, NEG
                    ).astype(np.float32)
    # ohd[k, h, m] = (m == h): one-hot column matmul lands head-h denominators
    # on psum partition h (M=128 so it stays in the fast matmul class)
    ohd = np.zeros((P, HPG, P), np.float32)
    for h in range(HPG):
        ohd[:, h, h] = 1.0
    ohd = ohd.reshape(P, HPG * P)
    ohbc = np.zeros((HPG, HPG, P), np.float32)
    for h in range(HPG):
        ohbc[h, h, :] = 1.0
    ohbc = ohbc.reshape(HPG, HPG * P)
    idn = np.eye(P, dtype=np.float32)

    in_maps = []
    for core in range(NCORES):
        b, g = divmod(core, HPG)
        xt = np.ascontiguousarray(x[b].T)
        wq = np.ascontiguousarray(Wq[:, g * HPG * HD:(g + 1) * HPG * HD])
        wk = np.ascontiguousarray(Wk[:, g * HD:(g + 1) * HD])
        wv = np.ascontiguousarray(Wv[:, g * HD:(g + 1) * HD])
        wo = np.ascontiguousarray(Wo[g * HPG * HD:(g + 1) * HPG * HD, :])
        in_maps.append({
            "xt": xt, "wq": wq, "wk": wk, "wv": wv, "wo": wo,
            "cosf": cosf, "sinf": sinf, "mtri": mtri,
            "ohd": ohd, "ohbc": ohbc, "idn": idn,
        })
    return in_maps


def run(x, rope_cache, Wq, Wk, Wv, Wo, trace=False, **kw):
    from concourse.bass_utils import run_bass_kernel_spmd

    if "nc" not in _CACHE:
        _CACHE["nc"] = _build()
    nc = _CACHE["nc"]
    in_maps = _host_inputs(x, rope_cache, Wq, Wk, Wv, Wo)
    res = run_bass_kernel_spmd(nc, in_maps, core_ids=list(range(NCORES)),
                               trace=trace, **kw)
    out = np.empty((B, T, C), np.float32)
    for b in range(B):
        acc = res.results[b * HPG]["o_part"].astype(np.float32).copy()
        for g in range(1, HPG):
            acc += res.results[b * HPG + g]["o_part"]
        out[b] = acc
    return out, res


def kernel(x, rope_cache, Wq, Wk, Wv, Wo):
    out, _ = run(x, rope_cache, Wq, Wk, Wv, Wo, trace=False)
    return out
